# revision 4
# baseline (speedup 1.0000x reference)
"""Trainium2 Bass kernel for nn_DictionaryWiseModel.

Reference computation (per notebook b):
    mask[c,l]  = src[b,c] <= l <= end[b,c]
    pooled     = (mask @ feature[b]) / counts          # [C, H]
    logits     = pooled @ fc_weight.T + fc_bias        # [C, 1]
Output: logits stacked over b -> [B*C, 1].

Strategy: data-parallel over B across 8 cores (1 notebook per core).
Per core the big einsum is done on the tensor engine as 16 accumulating
matmuls (span-mask chunk [128,64] stationary, feature chunk [128,512]x2
moving), masks are built on-chip with iota + two DVE compares, and the
final H-contraction with fc_weight is one fused tensor_tensor_reduce.
"""

import numpy as np

B, L, H, C = 8, 2048, 1024, 64
NCH = L // 128  # 16 l-chunks of 128

_CACHE = {}


def _build_nc():
    import concourse.bacc as bacc
    import concourse.mybir as mybir
    import concourse.tile as tile

    f32 = mybir.dt.float32
    i32 = mybir.dt.int32
    Alu = mybir.AluOpType

    nc = bacc.Bacc("TRN2", target_bir_lowering=False, debug=False)

    feat = nc.dram_tensor("feature", [L, H], f32, kind="ExternalInput")
    pos = nc.dram_tensor("pos", [C, 2], i32, kind="ExternalInput")
    fcw = nc.dram_tensor("fc_w", [1, H], f32, kind="ExternalInput")
    fcb = nc.dram_tensor("fc_b", [1, 1], f32, kind="ExternalInput")
    outd = nc.dram_tensor("out", [C, 1], f32, kind="ExternalOutput")

    with tile.TileContext(nc) as tc:
        with (
            tc.tile_pool(name="setup", bufs=1) as setup,
            tc.tile_pool(name="featp", bufs=4) as featp,
            tc.tile_pool(name="acc", bufs=1, space="PSUM") as accp,
            tc.tile_pool(name="bcast", bufs=1, space="PSUM") as bcastp,
        ):
            # ---- constants / position data ----
            ones = setup.tile([1, 128], f32)
            nc.vector.memset(ones[:], 1.0)

            pos_sb = setup.tile([C, 2], i32)
            nc.sync.dma_start(pos_sb[:], pos[:])

            # se row: [src(64) | end+1(64)] on one partition
            se_i = setup.tile([1, 128], i32)
            nc.sync.dma_start(se_i[:1, 0:C], pos[:, 0:1].rearrange("c o -> o c"))
            nc.sync.dma_start(se_i[:1, C : 2 * C], pos[:, 1:2].rearrange("c o -> o c"))
            nc.vector.tensor_scalar_add(se_i[:1, C : 2 * C], se_i[:1, C : 2 * C], 1)
            se_f = setup.tile([1, 128], f32)
            nc.vector.tensor_copy(se_f[:], se_i[:])

            # broadcast se row to all 128 partitions (outer product with ones)
            se_ps = bcastp.tile([128, 128], f32)
            nc.tensor.matmul(se_ps[:], ones[:1, :], se_f[:1, :], start=True, stop=True)
            se_sb = setup.tile([128, 128], f32)
            nc.vector.tensor_copy(se_sb[:], se_ps[:])

            # fc weight broadcast to C partitions
            w_sb = setup.tile([1, H], f32)
            nc.sync.dma_start(w_sb[:], fcw[:])
            w_ps = bcastp.tile([C, H], f32)
            nc.tensor.matmul(w_ps[:, 0:512], ones[:1, 0:C], w_sb[:1, 0:512], start=True, stop=True)
            nc.tensor.matmul(w_ps[:, 512:1024], ones[:1, 0:C], w_sb[:1, 512:1024], start=True, stop=True)
            w64 = setup.tile([C, H], f32)
            nc.vector.tensor_copy(w64[:], w_ps[:])

            # bias broadcast to [C, 1]
            b_sb = setup.tile([1, 1], f32)
            nc.sync.dma_start(b_sb[:], fcb[:])
            b_ps = bcastp.tile([C, 1], f32)
            nc.tensor.matmul(b_ps[:], ones[:1, 0:C], b_sb[:1, :], start=True, stop=True)

            # ---- span masks for all 16 chunks: [p, i, c] ----
            iota_t = setup.tile([128, NCH * 128], f32)
            iota_r = iota_t[:].rearrange("p (i f) -> p i f", i=NCH)
            nc.gpsimd.iota(
                iota_r,
                pattern=[[128, NCH], [0, 128]],
                base=0,
                channel_multiplier=1,
                allow_small_or_imprecise_dtypes=True,
            )
            ge_t = setup.tile([128, NCH * 128], f32)
            ge_r = ge_t[:].rearrange("p (i f) -> p i f", i=NCH)
            se_b = se_sb[:].rearrange("p (o f) -> p o f", o=1).broadcast_to((128, NCH, 128))
            nc.vector.tensor_tensor(ge_r, iota_r, se_b, Alu.is_ge)

            mask_t = setup.tile([128, NCH * C], f32)
            mask_r = mask_t[:].rearrange("p (i c) -> p i c", i=NCH)
            # (l >= src) - (l >= end+1)  ->  1 exactly on [src, end]
            nc.vector.tensor_tensor(
                mask_r, ge_r[:, :, 0:C], ge_r[:, :, C : 2 * C], Alu.subtract
            )

            # ---- main loop: pooled[c, h] += mask_i^T @ F_i ----
            pooled = accp.tile([C, H], f32)
            for i in range(NCH):
                ft = featp.tile([128, H], f32)
                nc.sync.dma_start(ft[:], feat[i * 128 : (i + 1) * 128, :])
                for half in range(2):
                    nc.tensor.matmul(
                        pooled[:, half * 512 : (half + 1) * 512],
                        mask_r[:, i, :],
                        ft[:, half * 512 : (half + 1) * 512],
                        start=(i == 0),
                        stop=(i == NCH - 1),
                    )

            # ---- epilogue: dot with w, divide by counts, add bias ----
            ttr_scratch = setup.tile([C, H], f32)
            s_sb = setup.tile([C, 1], f32)
            nc.vector.tensor_tensor(ttr_scratch[:], pooled[:], w64[:], Alu.mult)
            nc.vector.tensor_reduce(
                s_sb[:], ttr_scratch[:], mybir.AxisListType.X, Alu.add
            )

            cnt_i = setup.tile([C, 1], i32)
            nc.vector.tensor_tensor(cnt_i[:], pos_sb[:, 1:2], pos_sb[:, 0:1], Alu.subtract)
            nc.vector.tensor_scalar_add(cnt_i[:], cnt_i[:], 1)
            cnt_f = setup.tile([C, 1], f32)
            nc.vector.tensor_copy(cnt_f[:], cnt_i[:])

            rcp = setup.tile([C, 1], f32)
            nc.vector.reciprocal(rcp[:], cnt_f[:])
            res = setup.tile([C, 1], f32)
            # res = s * (1/cnt) + bias
            nc.vector.scalar_tensor_tensor(
                res[:], s_sb[:], rcp[:], b_ps[:], Alu.mult, Alu.add
            )

            nc.sync.dma_start(outd[:], res[:])

    nc.compile()
    return nc


def kernel(feature, fc_weight, fc_bias, position_list):
    from concourse import bass_utils

    feature = np.asarray(feature, dtype=np.float32)
    fc_weight = np.asarray(fc_weight, dtype=np.float32)
    fc_bias = np.asarray(fc_bias, dtype=np.float32).reshape(1, 1)
    position_list = np.asarray(position_list, dtype=np.int32)

    nc = _CACHE.get("nc")
    if nc is None:
        nc = _build_nc()
        _CACHE["nc"] = nc

    in_maps = [
        {
            "feature": np.ascontiguousarray(feature[b]),
            "pos": np.ascontiguousarray(position_list[b]),
            "fc_w": fc_weight,
            "fc_b": fc_bias,
        }
        for b in range(B)
    ]
    res = bass_utils.run_bass_kernel_spmd(nc, in_maps, list(range(B)))
    out = np.concatenate([res.results[b]["out"] for b in range(B)], axis=0)
    return out.astype(np.float32)


# revision 8
# speedup vs baseline: 1.1861x; 1.1861x over previous
"""Trainium2 Bass kernel for nn_DictionaryWiseModel.

Reference computation (per notebook b):
    mask[c,l]  = src[b,c] <= l <= end[b,c]
    pooled     = (mask @ feature[b]) / counts          # [C, H]
    logits     = pooled @ fc_weight.T + fc_bias        # [C, 1]
Output: logits stacked over b -> [B*C, 1].

Strategy: data-parallel over B across 8 cores (1 notebook per core).
Per core the big einsum is done on the tensor engine as 16 accumulating
matmuls (span-mask chunk [128,64] stationary, feature chunk [128,512]x2
moving), masks are built on-chip with iota + two DVE compares, and the
final H-contraction with fc_weight is one fused tensor_tensor_reduce.
"""

import numpy as np

B, L, H, C = 8, 2048, 1024, 64
NCH = L // 128  # 16 l-chunks of 128

_CACHE = {}


def _build_nc():
    import concourse.bacc as bacc
    import concourse.mybir as mybir
    import concourse.tile as tile

    f32 = mybir.dt.float32
    i32 = mybir.dt.int32
    Alu = mybir.AluOpType

    f32r = mybir.dt.float32r

    nc = bacc.Bacc("TRN2", target_bir_lowering=False, debug=False)

    # float32r: same bits as f32 on the host side, but the PE runs matmuls
    # at 1 cycle/row instead of 4.
    feat = nc.dram_tensor("feature", [L, H], f32r, kind="ExternalInput")
    pos = nc.dram_tensor("pos", [C, 2], i32, kind="ExternalInput")
    fcw = nc.dram_tensor("fc_w", [1, H], f32, kind="ExternalInput")
    fcb = nc.dram_tensor("fc_b", [1, 1], f32, kind="ExternalInput")
    outd = nc.dram_tensor("out", [C, 1], f32, kind="ExternalOutput")

    with tile.TileContext(nc) as tc:
        with (
            tc.tile_pool(name="setup", bufs=1) as setup,
            tc.tile_pool(name="featp", bufs=4) as featp,
            tc.tile_pool(name="acc", bufs=1, space="PSUM") as accp,
            tc.tile_pool(name="bcast", bufs=1, space="PSUM") as bcastp,
        ):
            # ---- constants / position data ----
            ones = setup.tile([1, 128], f32)
            nc.vector.memset(ones[:], 1.0)

            pos_sb = setup.tile([C, 2], i32)
            nc.sync.dma_start(pos_sb[:], pos[:])

            # se row: [src(64) | end+1(64)] on one partition
            se_i = setup.tile([1, 128], i32)
            nc.sync.dma_start(se_i[:1, 0:C], pos[:, 0:1].rearrange("c o -> o c"))
            nc.sync.dma_start(se_i[:1, C : 2 * C], pos[:, 1:2].rearrange("c o -> o c"))
            nc.vector.tensor_scalar_add(se_i[:1, C : 2 * C], se_i[:1, C : 2 * C], 1)
            se_f = setup.tile([1, 128], f32)
            nc.vector.tensor_copy(se_f[:], se_i[:])

            # broadcast se row to all 128 partitions (outer product with ones)
            se_ps = bcastp.tile([128, 128], f32)
            nc.tensor.matmul(se_ps[:], ones[:1, :], se_f[:1, :], start=True, stop=True)
            se_sb = setup.tile([128, 128], f32)
            nc.vector.tensor_copy(se_sb[:], se_ps[:])

            # fc weight broadcast to C partitions
            w_sb = setup.tile([1, H], f32)
            nc.sync.dma_start(w_sb[:], fcw[:])
            w_ps = bcastp.tile([C, H], f32)
            nc.tensor.matmul(w_ps[:, 0:512], ones[:1, 0:C], w_sb[:1, 0:512], start=True, stop=True)
            nc.tensor.matmul(w_ps[:, 512:1024], ones[:1, 0:C], w_sb[:1, 512:1024], start=True, stop=True)
            w64 = setup.tile([C, H], f32)
            nc.vector.tensor_copy(w64[:], w_ps[:])

            # bias broadcast to [C, 1]
            b_sb = setup.tile([1, 1], f32)
            nc.sync.dma_start(b_sb[:], fcb[:])
            b_ps = bcastp.tile([C, 1], f32)
            nc.tensor.matmul(b_ps[:], ones[:1, 0:C], b_sb[:1, :], start=True, stop=True)

            # ---- span masks for all 16 chunks: [p, i, c] ----
            iota_t = setup.tile([128, NCH * 128], f32)
            iota_r = iota_t[:].rearrange("p (i f) -> p i f", i=NCH)
            nc.gpsimd.iota(
                iota_r,
                pattern=[[128, NCH], [0, 128]],
                base=0,
                channel_multiplier=1,
                allow_small_or_imprecise_dtypes=True,
            )
            ge_t = setup.tile([128, NCH * 128], f32)
            ge_r = ge_t[:].rearrange("p (i f) -> p i f", i=NCH)
            se_b = se_sb[:].rearrange("p (o f) -> p o f", o=1).broadcast_to((128, NCH, 128))
            nc.vector.tensor_tensor(ge_r, iota_r, se_b, Alu.is_ge)

            mask_t = setup.tile([128, NCH * C], f32r)
            mask_r = mask_t[:].rearrange("p (i c) -> p i c", i=NCH)
            # (l >= src) - (l >= end+1)  ->  1 exactly on [src, end]
            nc.vector.tensor_tensor(
                mask_r, ge_r[:, :, 0:C], ge_r[:, :, C : 2 * C], Alu.subtract
            )

            # ---- main loop: pooled[c, h] += mask_i^T @ F_i ----
            # mask is exactly 0/1 so fp32r only rounds the feature mantissa.
            pooled = accp.tile([C, H], f32)
            for i in range(NCH):
                ft = featp.tile([128, H], f32r)
                nc.sync.dma_start(ft[:], feat[i * 128 : (i + 1) * 128, :])
                for half in range(2):
                    nc.tensor.matmul(
                        pooled[:, half * 512 : (half + 1) * 512],
                        mask_r[:, i, :],
                        ft[:, half * 512 : (half + 1) * 512],
                        start=(i == 0),
                        stop=(i == NCH - 1),
                    )

            # ---- epilogue: dot with w, divide by counts, add bias ----
            ttr_scratch = setup.tile([C, H], f32)
            s_sb = setup.tile([C, 1], f32)
            nc.vector.tensor_tensor(ttr_scratch[:], pooled[:], w64[:], Alu.mult)
            nc.vector.tensor_reduce(
                s_sb[:], ttr_scratch[:], mybir.AxisListType.X, Alu.add
            )

            cnt_i = setup.tile([C, 1], i32)
            nc.vector.tensor_tensor(cnt_i[:], pos_sb[:, 1:2], pos_sb[:, 0:1], Alu.subtract)
            nc.vector.tensor_scalar_add(cnt_i[:], cnt_i[:], 1)
            cnt_f = setup.tile([C, 1], f32)
            nc.vector.tensor_copy(cnt_f[:], cnt_i[:])

            rcp = setup.tile([C, 1], f32)
            nc.vector.reciprocal(rcp[:], cnt_f[:])
            res = setup.tile([C, 1], f32)
            # res = s * (1/cnt) + bias
            nc.vector.scalar_tensor_tensor(
                res[:], s_sb[:], rcp[:], b_ps[:], Alu.mult, Alu.add
            )

            nc.sync.dma_start(outd[:], res[:])

    nc.compile()
    return nc


def kernel(feature, fc_weight, fc_bias, position_list):
    from concourse import bass_utils

    feature = np.asarray(feature, dtype=np.float32)
    fc_weight = np.asarray(fc_weight, dtype=np.float32)
    fc_bias = np.asarray(fc_bias, dtype=np.float32).reshape(1, 1)
    position_list = np.asarray(position_list, dtype=np.int32)

    nc = _CACHE.get("nc")
    if nc is None:
        nc = _build_nc()
        _CACHE["nc"] = nc

    in_maps = [
        {
            "feature": np.ascontiguousarray(feature[b]),
            "pos": np.ascontiguousarray(position_list[b]),
            "fc_w": fc_weight,
            "fc_b": fc_bias,
        }
        for b in range(B)
    ]
    res = bass_utils.run_bass_kernel_spmd(nc, in_maps, list(range(B)))
    out = np.concatenate([res.results[b]["out"] for b in range(B)], axis=0)
    return out.astype(np.float32)
